# revision 6
# baseline (speedup 1.0000x reference)
"""Trainium2 Bass kernel for a dense transformer block (pre-LN, causal attn).

Sharding across 8 NeuronCores:
  - sequence-sharded: LN1, QKV projection, attn-output proj, LN2, MLP
    (core c owns rows [512c, 512c+512) of T=4096)
  - head-sharded: attention itself (core c owns heads 2c, 2c+1 over all T)
  - collectives: A2A#1 split into a qk collective (fired early) and a v
    collective that flies while the attention S matmuls start; A2A#2
    carries sender-normalized y in transposed orientation so the receiver
    assembles yT with two DMAs (no transposes, no reciprocals).

LayerNorm scale/bias are folded into the following matmul weights/biases
on the host, so the device LN is a single bn_stats/bn_aggr + one
activation pass. All matmuls run in bf16 with fp32 PSUM accumulation.
Softmax skips max-subtraction (scores bounded by construction).
"""

import sys

for _p in ("/opt/trn_rl_repo", "/root/.axon_site/_ro/trn_rl_repo"):
    if _p not in sys.path:
        sys.path.insert(0, _p)

import numpy as np
import ml_dtypes

import concourse.bass as bass
import concourse.mybir as mybir
import concourse.tile as tile
from concourse import bacc, bass_utils
from concourse.bass import ds, ts

F32 = mybir.dt.float32
BF16 = mybir.dt.bfloat16
AF = mybir.ActivationFunctionType
ALU = mybir.AluOpType

# model dims
D = 1024
T = 4096
H = 16
DH = 64
DFF = 4096
EPS = 1e-5
NCORES = 8
CHUNK = T // NCORES        # 512 rows per core
RG = CHUNK // 128          # 4 row groups
NQT = T // 128             # 32 key subblocks (for the 2 owned heads)
DT = D // 128              # 8 d-tiles
NFFT = DFF // 128          # 32 dff tiles
KB = 3                     # key-tile batch for one exp call

_cached = {}


def _build_nc():
    nc = bacc.Bacc("TRN2", target_bir_lowering=False)

    x_c = nc.dram_tensor("x_c", [CHUNK, D], F32, kind="ExternalInput")
    w_attn = nc.dram_tensor("w_attn", [D, 3 * D], BF16, kind="ExternalInput")
    w_proj = nc.dram_tensor("w_proj", [D, D], BF16, kind="ExternalInput")
    w_fc = nc.dram_tensor("w_fc", [D, DFF], BF16, kind="ExternalInput")
    w_fc2 = nc.dram_tensor("w_fc2", [DFF, D], BF16, kind="ExternalInput")
    b_qk = nc.dram_tensor("b_qk", [128, 16], F32, kind="ExternalInput")
    b_fc = nc.dram_tensor("b_fc", [128, NFFT], F32, kind="ExternalInput")
    bv_bc = nc.dram_tensor("bv_bc", [128, D], BF16, kind="ExternalInput")
    bproj_bc = nc.dram_tensor("bproj_bc", [128, D], F32, kind="ExternalInput")
    bfc2_bc = nc.dram_tensor("bfc2_bc", [128, D], F32, kind="ExternalInput")
    triu_in = nc.dram_tensor("triu", [128, 128], BF16, kind="ExternalInput")
    ident_in = nc.dram_tensor("ident", [128, 128], BF16, kind="ExternalInput")

    out_c = nc.dram_tensor("out_c", [CHUNK, D], F32, kind="ExternalOutput")

    x_v = x_c.rearrange("(rg p) d -> p rg d", p=128)
    out_v = out_c.rearrange("(rg p) d -> p rg d", p=128)
    wa_v = w_attn.rearrange("(dt p) c -> p dt c", p=128)
    wp_v = w_proj.rearrange("(dt p) c -> p dt c", p=128)
    wfc_v = w_fc.rearrange("(dt p) c -> p dt c", p=128)
    wfc2_v = w_fc2.rearrange("(ft p) c -> p ft c", p=128)

    with tile.TileContext(nc) as tc:
        with (
            tc.tile_pool(name="const", bufs=1) as const,
            tc.tile_pool(name="persist", bufs=1) as persist,
            tc.tile_pool(name="dram", bufs=1, space="DRAM") as dram,
        ):
            x_sb = persist.tile([128, RG, D], F32, tag="x_sb")
            for rg in range(RG):
                nc.sync.dma_start(x_sb[:, rg], x_v[:, rg])
            triu = const.tile([128, 128], BF16)
            ident = const.tile([128, 128], BF16)
            nc.sync.dma_start(triu[:], triu_in[:])
            nc.sync.dma_start(ident[:], ident_in[:])
            bv = const.tile([128, D], BF16)
            bqk_sb = const.tile([128, 16], F32)
            bfc_sb = const.tile([128, NFFT], F32)
            bproj = const.tile([128, D], F32)
            bfc2 = const.tile([128, D], F32)
            eps_sb = const.tile([128, 1], F32)
            zero_sb = const.tile([128, 1], F32)
            nc.vector.memset(eps_sb[:], EPS)
            nc.vector.memset(zero_sb[:], 0.0)
            nc.sync.dma_start(bv[:], bv_bc[:])
            nc.sync.dma_start(bqk_sb[:], b_qk[:])
            nc.sync.dma_start(bfc_sb[:], b_fc[:])
            nc.sync.dma_start(bproj[:], bproj_bc[:])
            nc.sync.dma_start(bfc2[:], bfc2_bc[:])

            def layernorm(pool, pt_pool, src_col, hT):
                """src_col: [128, RG, D] fp32 sbuf; writes hT [128, DT, CHUNK]
                bf16, normalized only (affine folded into weights)."""
                for rg in range(RG):
                    xin = src_col[:, rg]
                    st6 = pool.tile([128, 2, 6], F32, tag="ln_st6")
                    for g in range(2):
                        nc.vector.bn_stats(st6[:, g], xin[:, ds(g * 512, 512)])
                    mv = pool.tile([128, 2], F32, tag="ln_mv")
                    nc.vector.bn_aggr(mv[:], st6[:])
                    std = pool.tile([128, 1], F32, tag="ln_std")
                    nc.scalar.activation(std[:], mv[:, 1:2], AF.Sqrt, bias=eps_sb[:])
                    rstd = pool.tile([128, 1], F32, tag="ln_rstd")
                    nc.vector.reciprocal(rstd[:], std[:])
                    nmr = pool.tile([128, 1], F32, tag="ln_nmr")
                    nc.vector.scalar_tensor_tensor(
                        nmr[:], mv[:, 0:1], -1.0, rstd[:], ALU.mult, ALU.mult
                    )
                    h = pool.tile([128, D], BF16, tag="ln_h")
                    nc.scalar.activation(
                        h[:], xin, AF.Identity, bias=nmr[:], scale=rstd[:]
                    )
                    pt = pt_pool.tile([128, 1024], BF16, tag="pt")
                    for d in range(DT):
                        nc.tensor.transpose(pt[:, ts(d, 128)], h[:, ts(d, 128)], ident[:])
                    nc.scalar.activation(
                        hT[:, :, ds(rg * 128, 128)],
                        pt[:].rearrange("p (a b) -> p a b", a=DT),
                        AF.Copy,
                    )

            # ================= Phase A/B: LN1, QK -> A2A#1qk, V -> A2A#1v ====
            a2a1q_in = dram.tile([NCORES, 2, 128, 512], BF16)
            a2a1q_out = dram.tile([NCORES, 2, 128, 512], BF16)
            a2a1v_in = dram.tile([NCORES, 128, 512], BF16)
            a2a1v_out = dram.tile([NCORES, 128, 512], BF16)
            with (
                tc.tile_pool(name="ph_a", bufs=2) as ph_a,
                tc.tile_pool(name="ps1", bufs=3, space="PSUM") as ps1,
                tc.tile_pool(name="pspt1", bufs=2, space="PSUM") as pspt1,
            ):
                hT = ph_a.tile([128, DT, CHUNK], BF16, tag="hT", bufs=1)
                layernorm(ph_a, pspt1, x_sb, hT)

                qkT = ph_a.tile([128, 16, CHUNK], BF16, tag="qkT", bufs=1)
                for o in range(16):
                    w_t = ph_a.tile([128, DT, 128], BF16, tag="w_qk", bufs=4)
                    nc.sync.dma_start(w_t[:], wa_v[:, :, ds(o * 128, 128)])
                    ps = ps1.tile([128, 512], F32, tag="mm")
                    for d in range(DT):
                        nc.tensor.matmul(
                            ps[:], w_t[:, d], hT[:, d],
                            start=(d == 0), stop=(d == DT - 1),
                        )
                    nc.scalar.activation(
                        qkT[:, o], ps[:], AF.Identity, bias=bqk_sb[:, o : o + 1]
                    )
                    nc.sync.dma_start(a2a1q_in[o % 8, o // 8], qkT[:, o])
                nc.gpsimd.collective_compute(
                    "AllToAll",
                    ALU.bypass,
                    ins=[a2a1q_in.opt()],
                    outs=[a2a1q_out.opt()],
                    replica_groups=[list(range(NCORES))],
                )

                # v while the qk collective flies
                v_nat = ph_a.tile([128, RG, D], BF16, tag="v_nat", bufs=1)
                wv_t = ph_a.tile([128, DT, 1024], BF16, tag="w_v", bufs=1)
                nc.sync.dma_start(wv_t[:], wa_v[:, :, ds(2 * D, 1024)])
                for rg in range(RG):
                    for vh in range(2):
                        ps = ps1.tile([128, 512], F32, tag="mm")
                        for d in range(DT):
                            nc.tensor.matmul(
                                ps[:], hT[:, d, ds(rg * 128, 128)],
                                wv_t[:, d, ds(vh * 512, 512)],
                                start=(d == 0), stop=(d == DT - 1),
                            )
                        nc.vector.tensor_tensor(
                            v_nat[:, rg, ds(vh * 512, 512)], ps[:],
                            bv[:, ds(vh * 512, 512)], ALU.add,
                        )
                for j in range(NCORES):
                    nc.sync.dma_start(
                        a2a1v_in[j].rearrange("p (rg w) -> p rg w", rg=RG),
                        v_nat[:, :, ds(j * 128, 128)],
                    )
                nc.gpsimd.collective_compute(
                    "AllToAll",
                    ALU.bypass,
                    ins=[a2a1v_in.opt()],
                    outs=[a2a1v_out.opt()],
                    replica_groups=[list(range(NCORES))],
                )

            # ============ Phase D: attention (2 owned heads, all T) ==========
            # Per dest chunk qg (descending, big first): all S^T tiles + exp,
            # then PV with lhsT=[V|1] giving unnormalized yT + denominator
            # row; normalize on the sender (reciprocal + partition broadcast)
            # so A2A#2 carries ready-to-use yT slices.
            a2a2_in = dram.tile([NCORES, 2, 64, 512], BF16)
            a2a2_out = dram.tile([NCORES, 2, 64, 512], BF16)
            wp_sb = persist.tile([128, DT, 1024], BF16, tag="wp_sb")
            with (
                tc.tile_pool(name="ph_d", bufs=3) as ph_d,
                tc.tile_pool(name="ps_s", bufs=2, space="PSUM") as ps_s,
                tc.tile_pool(name="ps_y", bufs=1, space="PSUM") as ps_y,
            ):
                qT = ph_d.tile([128, T], BF16, tag="qT", bufs=1)
                kT = ph_d.tile([128, T], BF16, tag="kT", bufs=1)
                nc.sync.dma_start(
                    qT[:].rearrange("p (r w) -> p r w", r=NCORES),
                    a2a1q_out[:, 0].rearrange("r p w -> p r w"),
                )
                nc.sync.dma_start(
                    kT[:].rearrange("p (r w) -> p r w", r=NCORES),
                    a2a1q_out[:, 1].rearrange("r p w -> p r w"),
                )
                vh_sb = [
                    ph_d.tile([128, NQT, 65], BF16, tag=f"v_h{hh}", bufs=1,
                              name=f"v_h{hh}")
                    for hh in range(2)
                ]
                for hh in range(2):
                    nc.vector.memset(vh_sb[hh][:, :, 64:65], 1.0)
                    for r in range(NCORES):
                        nc.sync.dma_start(
                            vh_sb[hh][:, ds(r * RG, RG), 0:64],
                            a2a1v_out[r].rearrange(
                                "p (rg hh dh) -> p rg hh dh", rg=RG, hh=2
                            )[:, :, hh],
                        )
                # prefetch proj weights during attention
                nc.sync.dma_start(wp_sb[:], wp_v[:])

                qT_h = [qT[ds(hh * 64, 64), :] for hh in range(2)]
                kT_h = [kT[ds(hh * 64, 64), :] for hh in range(2)]
                for qg in range(NCORES - 1, -1, -1):
                    nkt = 4 * qg + 4
                    for hh in range(2):
                        et = ph_d.tile(
                            [128, nkt * 512], BF16, tag="et", bufs=2,
                            name=f"et_{qg}_{hh}",
                        )
                        for kb in range((nkt + KB - 1) // KB):
                            nt = min(KB, nkt - kb * KB)
                            st_ps = ps_s.tile([128, KB * 512], F32, tag="s")
                            for t in range(nt):
                                kt = kb * KB + t
                                nc.tensor.matmul(
                                    st_ps[:, ts(t, 512)],
                                    kT_h[hh][:, ds(kt * 128, 128)],
                                    qT_h[hh][:, ds(qg * 512, 512)],
                                    start=True, stop=True,
                                )
                            nc.scalar.activation(
                                et[:, ds(kb * KB * 512, nt * 512)],
                                st_ps[:, : nt * 512], AF.Exp,
                                bias=zero_sb[:], scale=0.125,
                            )
                        for sl in range(4):
                            kt = 4 * qg + sl
                            nc.vector.tensor_tensor(
                                et[:, ds(kt * 512 + sl * 128, 128)],
                                et[:, ds(kt * 512 + sl * 128, 128)],
                                triu[:], ALU.mult,
                            )
                            for z in range(sl):
                                nc.gpsimd.memset(
                                    et[:, ds(kt * 512 + z * 128, 128)], 0.0
                                )
                        y_ps = ps_y.tile(
                            [128, 512], F32, tag=f"y{hh}", bufs=1,
                            name=f"y{hh}_{qg}",
                        )
                        for kt in range(nkt):
                            nc.tensor.matmul(
                                y_ps[:65, :],
                                vh_sb[hh][:, kt],
                                et[:, ts(kt, 512)],
                                start=(kt == 0), stop=(kt == nkt - 1),
                            )
                        rden = ph_d.tile([1, 512], F32, tag="rden")
                        nc.vector.reciprocal(rden[:], y_ps[64:65, :])
                        dbc = ph_d.tile([64, 512], F32, tag="dbc")
                        nc.gpsimd.partition_broadcast(dbc[:], rden[:])
                        yn = ph_d.tile([64, 512], BF16, tag="yn")
                        nc.vector.tensor_tensor(
                            yn[:], y_ps[0:64, :], dbc[:], ALU.mult
                        )
                        nc.sync.dma_start(a2a2_in[qg, hh], yn[:])
                nc.gpsimd.collective_compute(
                    "AllToAll",
                    ALU.bypass,
                    ins=[a2a2_in.opt()],
                    outs=[a2a2_out.opt()],
                    replica_groups=[list(range(NCORES))],
                )

            # ============ Phase E: proj, LN2 ================================
            with (
                tc.tile_pool(name="ph_e", bufs=2) as ph_e,
                tc.tile_pool(name="ps2", bufs=3, space="PSUM") as ps2,
                tc.tile_pool(name="pspt2", bufs=2, space="PSUM") as pspt2,
            ):
                yT = ph_e.tile([128, DT, CHUNK], BF16, tag="yT", bufs=1)
                nc.sync.dma_start(
                    yT[0:64, :, :],
                    a2a2_out[:, 0].rearrange("r p w -> p r w"),
                )
                nc.sync.dma_start(
                    yT[64:128, :, :],
                    a2a2_out[:, 1].rearrange("r p w -> p r w"),
                )

                x2_sb = persist.tile([128, RG, D], F32, tag="x2")
                for rg in range(RG):
                    for half in range(2):
                        ps = ps2.tile([128, 512], F32, tag="mm")
                        for d in range(DT):
                            nc.tensor.matmul(
                                ps[:], yT[:, d, ds(rg * 128, 128)],
                                wp_sb[:, d, ds(half * 512, 512)],
                                start=(d == 0), stop=(d == DT - 1),
                            )
                        tmp = ph_e.tile([128, 512], F32, tag="proj_tmp")
                        nc.vector.tensor_tensor(
                            tmp[:], ps[:], bproj[:, ds(half * 512, 512)], ALU.add
                        )
                        nc.vector.tensor_tensor(
                            x2_sb[:, rg, ds(half * 512, 512)], tmp[:],
                            x_sb[:, rg, ds(half * 512, 512)], ALU.add,
                        )

                h2T = persist.tile([128, DT, CHUNK], BF16, tag="h2T")
                layernorm(ph_e, pspt2, x2_sb, h2T)

            # ============ Phase F: MLP, output ==============================
            with (
                tc.tile_pool(name="ph_f", bufs=2) as ph_f,
                tc.tile_pool(name="ps3", bufs=3, space="PSUM") as ps3,
                tc.tile_pool(name="ps_acc", bufs=1, space="PSUM") as ps_acc,
            ):
                mT = ph_f.tile([128, NFFT, CHUNK], BF16, tag="mT", bufs=1)
                for ft in range(NFFT):
                    w_t = ph_f.tile([128, DT, 128], BF16, tag="w_fc", bufs=4)
                    nc.sync.dma_start(w_t[:], wfc_v[:, :, ds(ft * 128, 128)])
                    ps = ps3.tile([128, 512], F32, tag="mm")
                    for d in range(DT):
                        nc.tensor.matmul(
                            ps[:], w_t[:, d], h2T[:, d],
                            start=(d == 0), stop=(d == DT - 1),
                        )
                    nc.scalar.activation(
                        mT[:, ft], ps[:], AF.Gelu_apprx_tanh,
                        bias=bfc_sb[:, ft : ft + 1],
                    )

                out_sb = ph_f.tile([128, RG, D], F32, tag="out_sb", bufs=1)
                for half in range(2):
                    acc = [
                        ps_acc.tile([128, 512], F32, tag=f"ps_o{rg}",
                                    name=f"ps_o{rg}_{half}")
                        for rg in range(RG)
                    ]
                    for ft in range(NFFT):
                        w_t = ph_f.tile([128, 512], BF16, tag="w_fc2", bufs=4)
                        nc.sync.dma_start(
                            w_t[:], wfc2_v[:, ft, ds(half * 512, 512)]
                        )
                        for rg in range(RG):
                            nc.tensor.matmul(
                                acc[rg][:], mT[:, ft, ds(rg * 128, 128)], w_t[:],
                                start=(ft == 0), stop=(ft == NFFT - 1),
                            )
                    for rg in range(RG):
                        tmp = ph_f.tile([128, 512], F32, tag="o_tmp")
                        nc.vector.tensor_tensor(
                            tmp[:], acc[rg][:], bfc2[:, ds(half * 512, 512)], ALU.add
                        )
                        nc.vector.tensor_tensor(
                            out_sb[:, rg, ds(half * 512, 512)], tmp[:],
                            x2_sb[:, rg, ds(half * 512, 512)], ALU.add,
                        )
                        nc.sync.dma_start(
                            out_v[:, rg, ds(half * 512, 512)],
                            out_sb[:, rg, ds(half * 512, 512)],
                        )

    nc.compile()
    return nc


def _prep_inputs(inputs):
    """Host-side shard + cast + LN affine folding. Returns per-core in_maps."""
    bf = ml_dtypes.bfloat16
    x = np.asarray(inputs["x"], np.float32).reshape(T, D)
    ln1s = np.asarray(inputs["ln1_scale"], np.float32)
    ln1b = np.asarray(inputs["ln1_bias"], np.float32)
    ln2s = np.asarray(inputs["ln2_scale"], np.float32)
    ln2b = np.asarray(inputs["ln2_bias"], np.float32)
    w_attn_f = np.asarray(inputs["W_attn"], np.float32)
    w_fc_f = np.asarray(inputs["W_fc"], np.float32)
    b_attn_f = np.asarray(inputs["b_attn"], np.float32) + ln1b @ w_attn_f
    b_fc_f = np.asarray(inputs["b_fc"], np.float32) + ln2b @ w_fc_f

    w_attn = (ln1s[:, None] * w_attn_f).astype(bf)
    w_fc = (ln2s[:, None] * w_fc_f).astype(bf)
    w_proj = np.asarray(inputs["W_proj"], np.float32).astype(bf)
    w_fc2 = np.asarray(inputs["W_fc2"], np.float32).astype(bf)
    b_qk = np.ascontiguousarray(b_attn_f[: 2 * D].reshape(16, 128).T)
    bv_bc = np.broadcast_to(b_attn_f[2 * D :].astype(bf), (128, D)).copy()
    b_fc = np.ascontiguousarray(b_fc_f.reshape(NFFT, 128).T)
    bproj = np.broadcast_to(np.asarray(inputs["b_proj"], np.float32), (128, D)).copy()
    bfc2 = np.broadcast_to(np.asarray(inputs["b_fc2"], np.float32), (128, D)).copy()
    triu = np.triu(np.ones((128, 128), np.float32)).astype(bf)
    ident = np.eye(128, dtype=np.float32).astype(bf)

    shared = dict(
        w_attn=w_attn, w_proj=w_proj, w_fc=w_fc, w_fc2=w_fc2,
        b_qk=b_qk, b_fc=b_fc, bv_bc=bv_bc,
        bproj_bc=bproj, bfc2_bc=bfc2, triu=triu, ident=ident,
    )
    return [
        {"x_c": np.ascontiguousarray(x[c * CHUNK : (c + 1) * CHUNK]), **shared}
        for c in range(NCORES)
    ]


def kernel(**inputs) -> np.ndarray:
    if "nc" not in _cached:
        _cached["nc"] = _build_nc()
    nc = _cached["nc"]
    in_maps = _prep_inputs(inputs)
    res = bass_utils.run_bass_kernel_spmd(
        nc, in_maps, core_ids=list(range(NCORES))
    )
    out = np.concatenate(
        [res.results[c]["out_c"] for c in range(NCORES)], axis=0
    )
    return out.reshape(1, T, D).astype(np.float32)
